# revision 3
# baseline (speedup 1.0000x reference)
"""Trainium2 Bass kernel for nn_Discriminator_44779329028358 (segment_reduce).

Math (per batch b):
    doc_proj = doc_encoding[b] @ W + bias            # [S, D]
    hard span: s_h = argmax(score_soft[0,b]), e_h = argmax(score_soft[1,b])
    gold span: (s_g, e_g) = answer_idx[:, b]
    answer[b]      = sum_{s_h<=s<=e_h} doc_proj[s]   # zeros when s_h > e_h
    answer_gold[b] = sum_{s_g<=s<=e_g} doc_proj[s]

Kernel strategy: masked span-sum over raw doc_encoding first (matmul with a
0/1 mask as the stationary operand, streaming doc tiles), then a single tiny
projection through W:  (mask @ doc) @ W == mask @ (doc @ W).  The count*bias
term is added on the host from the (returned) span indices; bias is zeros for
this problem's inputs anyway.

Sharding: data-parallel over batch across 8 cores (8 batches/core).
"""

from contextlib import ExitStack

import numpy as np

import concourse.bacc as bacc
import concourse.mybir as mybir
import concourse.tile as tile
from concourse.bass_utils import run_bass_kernel_spmd
from concourse.masks import make_identity

N_CORES = 8
B, S, D = 64, 2048, 256
BPC = B // N_CORES          # batches per core = 8
KT = S // 128               # k-tiles per batch = 16
F32 = mybir.dt.float32
F32R = mybir.dt.float32r

_cache = {}


def _emit(ctx, tc, doc, scores, gold, w, out, idx_out):
    nc = tc.nc

    const = ctx.enter_context(tc.tile_pool(name="const", bufs=1))
    small = ctx.enter_context(tc.tile_pool(name="small", bufs=1))
    docp = ctx.enter_context(tc.tile_pool(name="doc", bufs=3))
    span_ps = ctx.enter_context(tc.tile_pool(name="span_ps", bufs=2, space="PSUM"))
    aux_ps = ctx.enter_context(tc.tile_pool(name="aux_ps", bufs=1, space="PSUM"))
    ans_ps = ctx.enter_context(tc.tile_pool(name="ans_ps", bufs=1, space="PSUM"))

    # ---- constants ----
    w_sb = const.tile([128, 2, D], F32R)          # W rows (k2*128+p) -> [p, k2, d]
    nc.gpsimd.dma_start(w_sb[:], w.rearrange("(k2 p) d -> p k2 d", p=128))

    ident2 = const.tile([2, 2], F32)
    make_identity(nc, ident2[:])

    iota_i = const.tile([128, KT], mybir.dt.int32)   # val[p, kt] = p*KT + kt
    nc.gpsimd.iota(iota_i[:], pattern=[[1, KT]], base=0, channel_multiplier=KT)
    iota_f = const.tile([128, KT], F32)
    nc.vector.tensor_copy(iota_f[:], iota_i[:])

    # ---- scores + argmax ----
    # 16 problems on 16 partitions: p = t*BPC + b  (t: 0=start scores, 1=end)
    sc = small.tile([2 * BPC, S], F32)
    nc.gpsimd.dma_start(sc[:], scores.rearrange("t b s -> (t b) s"))

    mx = small.tile([2 * BPC, 8], F32)
    mi = small.tile([2 * BPC, 8], mybir.dt.uint32)
    nc.vector.max(mx[:], sc[:])
    nc.vector.max_index(mi[:], mx[:], sc[:])
    idxf = small.tile([2 * BPC, 1], F32)
    nc.vector.tensor_copy(idxf[:], mi[:, 0:1])       # uint32 -> f32 convert
    nc.gpsimd.dma_start(idx_out, idxf[:])

    # ---- index row, free-axis layout [1, 32]: cols 0:8 hs, 8:16 gs, 16:24 he, 24:32 ge
    idx_row = small.tile([1, 32], F32)
    nc.gpsimd.dma_start(idx_row[0:1, 0:BPC], idxf[0:BPC, 0:1])          # hard s
    nc.gpsimd.dma_start(idx_row[0:1, 16:16 + BPC], idxf[BPC:2 * BPC, 0:1])  # hard e
    nc.gpsimd.dma_start(idx_row[0:1, 8:8 + BPC], gold[0:1, :])          # gold s
    nc.gpsimd.dma_start(idx_row[0:1, 24:24 + BPC], gold[1:2, :])        # gold e

    idx_bc = small.tile([128, 32], F32)
    nc.gpsimd.partition_broadcast(idx_bc[:], idx_row[:])

    # ---- masks: mask[p, kt, b, t'] = (pos >= s) & (pos <= e), pos = p*KT + kt
    # column order per (kt): j = 2*b + t'  (t'=0 hard, 1 gold)
    s_view = idx_bc[:, 0:16].rearrange("p (t b) -> p t b", t=2).transpose([0, 2, 1])
    e_view = idx_bc[:, 16:32].rearrange("p (t b) -> p t b", t=2).transpose([0, 2, 1])
    iota_b = iota_f[:, :, None, None].broadcast_to([128, KT, BPC, 2])

    ge = small.tile([128, KT, BPC, 2], F32)
    le = small.tile([128, KT, BPC, 2], F32)
    mask = small.tile([128, KT, BPC, 2], F32R)
    nc.vector.tensor_tensor(ge[:], iota_b, s_view[:, None, :, :].broadcast_to([128, KT, BPC, 2]), mybir.AluOpType.is_ge)
    nc.vector.tensor_tensor(le[:], iota_b, e_view[:, None, :, :].broadcast_to([128, KT, BPC, 2]), mybir.AluOpType.is_le)
    nc.vector.tensor_tensor(mask[:], ge[:], le[:], mybir.AluOpType.mult)

    # ---- per-batch span sums + transposes ----
    # spanT collects transposed span sums: col 4*b + 2*k2 + t'
    spanT = aux_ps.tile([128, 4 * BPC], F32)
    lhsT = small.tile([128, 4 * BPC], F32R)

    for b in range(BPC):
        dt_ = docp.tile([128, KT, D], F32R)
        # doc rows r = p*KT + kt; contiguous 16 KiB per partition
        nc.sync.dma_start(dt_[:], doc[b].rearrange("(p x) d -> p x d", p=128))

        ps = span_ps.tile([2, D], F32)
        for kt in range(KT):
            nc.tensor.matmul(
                ps[:],
                mask[:, kt, b, :],
                dt_[:, kt, :],
                start=(kt == 0),
                stop=(kt == KT - 1),
            )
        sb = small.tile([2, D], F32, tag="span_sb")
        nc.vector.tensor_copy(sb[:], ps[:])
        # transpose [2, 128-half] -> [128, 2] into spanT columns
        # layout: col = 16*k2 + 2*b + t'  (k-half-major, contiguous per half)
        for k2 in range(2):
            c0 = 16 * k2 + 2 * b
            nc.tensor.transpose(spanT[:, c0:c0 + 2],
                                sb[:, 128 * k2:128 * (k2 + 1)], ident2[:])

    nc.vector.tensor_copy(lhsT[:], spanT[:])

    # ---- final projection: ans[j=(b,t'), d] = sum_k spanT[k, j] * W[k, d]
    ap = ans_ps.tile([2 * BPC, D], F32)
    for k2 in range(2):
        nc.tensor.matmul(
            ap[:],
            lhsT[:, 16 * k2:16 * (k2 + 1)],
            w_sb[:, k2, :],
            start=(k2 == 0),
            stop=(k2 == 1),
        )
    ans_sb = small.tile([2 * BPC, D], F32)
    nc.vector.tensor_copy(ans_sb[:], ap[:])
    nc.sync.dma_start(out, ans_sb[:])


def _build():
    nc = bacc.Bacc("TRN2", target_bir_lowering=False, debug=False,
                   num_devices=N_CORES)
    doc = nc.dram_tensor("doc", [BPC, S, D], F32R, kind="ExternalInput").ap()
    scores = nc.dram_tensor("scores", [2, BPC, S], F32, kind="ExternalInput").ap()
    gold = nc.dram_tensor("gold", [2, BPC], F32, kind="ExternalInput").ap()
    w = nc.dram_tensor("w", [D, D], F32R, kind="ExternalInput").ap()
    out = nc.dram_tensor("out", [2 * BPC, D], F32, kind="ExternalOutput").ap()
    idx_out = nc.dram_tensor("idx_out", [2 * BPC, 1], F32, kind="ExternalOutput").ap()

    with tile.TileContext(nc) as tc, ExitStack() as ctx:
        _emit(ctx, tc, doc, scores, gold, w, out, idx_out)
    nc.compile()
    return nc


def get_nc():
    if "nc" not in _cache:
        _cache["nc"] = _build()
    return _cache["nc"]


def make_in_maps(doc_encoding, score_soft, answer_idx, W_mlp):
    doc_encoding = np.asarray(doc_encoding, dtype=np.float32)
    score_soft = np.asarray(score_soft, dtype=np.float32)
    gold_f = np.asarray(answer_idx).astype(np.float32)
    w = np.ascontiguousarray(np.asarray(W_mlp, dtype=np.float32))
    maps = []
    for c in range(N_CORES):
        sl = slice(c * BPC, (c + 1) * BPC)
        maps.append({
            "doc": np.ascontiguousarray(doc_encoding[sl]),
            "scores": np.ascontiguousarray(score_soft[:, sl]),
            "gold": np.ascontiguousarray(gold_f[:, sl]),
            "w": w,
        })
    return maps


def assemble(results, answer_idx, b_mlp):
    """results: list of 8 per-core dicts with 'out' [16, D] and 'idx_out' [16, 1]."""
    b_mlp = np.asarray(b_mlp, dtype=np.float32)
    ans = np.empty((B, D), np.float32)
    goldo = np.empty((B, D), np.float32)
    for c in range(N_CORES):
        sl = slice(c * BPC, (c + 1) * BPC)
        o = np.asarray(results[c]["out"]).reshape(BPC, 2, D)
        ans[sl] = o[:, 0]
        goldo[sl] = o[:, 1]
        if b_mlp.any():
            idx = np.asarray(results[c]["idx_out"]).reshape(2 * BPC)
            cnt_h = np.maximum(0.0, idx[BPC:] - idx[:BPC] + 1.0)
            gi = np.asarray(answer_idx)[:, sl].astype(np.float64)
            cnt_g = np.maximum(0.0, gi[1] - gi[0] + 1.0)
            ans[sl] += cnt_h[:, None].astype(np.float32) * b_mlp[None, :]
            goldo[sl] += cnt_g[:, None].astype(np.float32) * b_mlp[None, :]
    return ans, goldo


def kernel(doc_encoding, score_soft, answer_idx, W_mlp, b_mlp):
    nc = get_nc()
    in_maps = make_in_maps(doc_encoding, score_soft, answer_idx, W_mlp)
    res = run_bass_kernel_spmd(nc, in_maps, list(range(N_CORES))).results
    return assemble(res, answer_idx, b_mlp)


# revision 5
# speedup vs baseline: 1.3021x; 1.3021x over previous
"""Trainium2 Bass kernel for nn_Discriminator_44779329028358 (segment_reduce).

Math (per batch b):
    doc_proj = doc_encoding[b] @ W + bias            # [S, D]
    hard span: s_h = argmax(score_soft[0,b]), e_h = argmax(score_soft[1,b])
    gold span: (s_g, e_g) = answer_idx[:, b]
    answer[b]      = sum_{s_h<=s<=e_h} doc_proj[s]   # zeros when s_h > e_h
    answer_gold[b] = sum_{s_g<=s<=e_g} doc_proj[s]

Kernel strategy: masked span-sum over raw doc_encoding first (matmul with a
0/1 mask as the stationary operand, streaming doc tiles), then a single tiny
projection through W:  (mask @ doc) @ W == mask @ (doc @ W).  The count*bias
term is added on the host from the (returned) span indices; bias is zeros for
this problem's inputs anyway.

Sharding: data-parallel over batch across 8 cores (8 batches/core).
"""

from contextlib import ExitStack

import numpy as np

import concourse.bacc as bacc
import concourse.mybir as mybir
import concourse.tile as tile
from concourse.bass_utils import run_bass_kernel_spmd
from concourse.masks import make_identity

N_CORES = 8
B, S, D = 64, 2048, 256
BPC = B // N_CORES          # batches per core = 8
KT = S // 128               # k-tiles per batch = 16
F32 = mybir.dt.float32
F32R = mybir.dt.float32r

_cache = {}


def _emit(ctx, tc, doc, scores, gold, w, out, idx_out):
    nc = tc.nc

    const = ctx.enter_context(tc.tile_pool(name="const", bufs=1))
    small = ctx.enter_context(tc.tile_pool(name="small", bufs=1))
    docp = ctx.enter_context(tc.tile_pool(name="doc", bufs=4))
    span_ps = ctx.enter_context(tc.tile_pool(name="span_ps", bufs=2, space="PSUM"))
    aux_ps = ctx.enter_context(tc.tile_pool(name="aux_ps", bufs=1, space="PSUM"))
    ans_ps = ctx.enter_context(tc.tile_pool(name="ans_ps", bufs=1, space="PSUM"))

    # All small DMAs ride the scalar (ACT) HWDGE ring so they never queue
    # behind the 16 MiB doc stream on the sync (SP) ring; no gpsimd (SWDGE)
    # DMAs anywhere so the broadcast path never waits on a Q7 drain.

    # ---- scores + argmax (critical path for the masks) ----
    # 16 problems on 16 partitions: p = t*BPC + b  (t: 0=start scores, 1=end)
    sc = small.tile([2 * BPC, S], F32)
    nc.scalar.dma_start(sc[:], scores.rearrange("t b s -> (t b) s"))

    # ---- index row, free-axis layout [1, 32]: cols 0:8 hs, 8:16 gs, 16:24 he, 24:32 ge
    idx_row = small.tile([1, 32], F32)
    nc.scalar.dma_start(idx_row[0:1, 8:8 + BPC], gold[0:1, :])          # gold s
    nc.scalar.dma_start(idx_row[0:1, 24:24 + BPC], gold[1:2, :])        # gold e

    # ---- constants ----
    w_sb = const.tile([128, 2, D], F32R)          # W rows (k2*128+p) -> [p, k2, d]
    nc.scalar.dma_start(w_sb[:], w.rearrange("(k2 p) d -> p k2 d", p=128))

    ident2 = const.tile([2, 2], F32)
    make_identity(nc, ident2[:])
    ident16 = const.tile([16, 16], F32)
    make_identity(nc, ident16[:])

    iota_i = const.tile([128, KT], mybir.dt.int32)   # val[p, kt] = p*KT + kt
    nc.gpsimd.iota(iota_i[:], pattern=[[1, KT]], base=0, channel_multiplier=KT)
    iota_f = const.tile([128, KT], F32)
    nc.vector.tensor_copy(iota_f[:], iota_i[:])

    mx = small.tile([2 * BPC, 8], F32)
    mi = small.tile([2 * BPC, 8], mybir.dt.uint32)
    nc.vector.max(mx[:], sc[:])
    nc.vector.max_index(mi[:], mx[:], sc[:])
    idxf = small.tile([2 * BPC, 1], F32)
    nc.vector.tensor_copy(idxf[:], mi[:, 0:1])       # uint32 -> f32 convert
    nc.scalar.dma_start(idx_out, idxf[:])

    # hard indices partition->free via PE transpose: [16,1] -> [1,16]
    idxT_ps = aux_ps.tile([1, 16], F32, tag="idxT")
    nc.tensor.transpose(idxT_ps[:], idxf[:], ident16[:])
    # psum cols 0:8 = hard s -> idx_row 0:8; cols 8:16 = hard e -> idx_row 16:24
    nc.vector.tensor_copy(
        idx_row[0:1, 0:32].rearrange("o (t x) -> o t x", t=2)[:, :, 0:BPC],
        idxT_ps[0:1, :].rearrange("o (t b) -> o t b", t=2),
    )

    idx_bc = small.tile([128, 32], F32)
    nc.gpsimd.partition_broadcast(idx_bc[:], idx_row[:])

    # ---- masks: mask[p, kt, b, t'] = (pos >= s) & (pos <= e), pos = p*KT + kt
    # column order per (kt): j = 2*b + t'  (t'=0 hard, 1 gold)
    s_view = idx_bc[:, 0:16].rearrange("p (t b) -> p t b", t=2).transpose([0, 2, 1])
    e_view = idx_bc[:, 16:32].rearrange("p (t b) -> p t b", t=2).transpose([0, 2, 1])
    iota_b = iota_f[:, :, None, None].broadcast_to([128, KT, BPC, 2])

    ge = small.tile([128, KT, BPC, 2], F32)
    le = small.tile([128, KT, BPC, 2], F32)
    mask = small.tile([128, KT, BPC, 2], F32R)
    nc.vector.tensor_tensor(ge[:], iota_b, s_view[:, None, :, :].broadcast_to([128, KT, BPC, 2]), mybir.AluOpType.is_ge)
    nc.vector.tensor_tensor(le[:], iota_b, e_view[:, None, :, :].broadcast_to([128, KT, BPC, 2]), mybir.AluOpType.is_le)
    nc.vector.tensor_tensor(mask[:], ge[:], le[:], mybir.AluOpType.mult)

    # ---- per-batch span sums + transposes ----
    # spanT collects transposed span sums: col 4*b + 2*k2 + t'
    spanT = aux_ps.tile([128, 4 * BPC], F32)
    lhsT = small.tile([128, 4 * BPC], F32R)

    for b in range(BPC):
        dt_ = docp.tile([128, KT, D], F32R)
        # doc rows r = p*KT + kt; contiguous 16 KiB per partition
        nc.sync.dma_start(dt_[:], doc[b].rearrange("(p x) d -> p x d", p=128))

        ps = span_ps.tile([2, D], F32)
        for kt in range(KT):
            nc.tensor.matmul(
                ps[:],
                mask[:, kt, b, :],
                dt_[:, kt, :],
                start=(kt == 0),
                stop=(kt == KT - 1),
            )
        sb = small.tile([2, D], F32, tag="span_sb")
        nc.vector.tensor_copy(sb[:], ps[:])
        # transpose [2, 128-half] -> [128, 2] into spanT columns
        # layout: col = 16*k2 + 2*b + t'  (k-half-major, contiguous per half)
        for k2 in range(2):
            c0 = 16 * k2 + 2 * b
            nc.tensor.transpose(spanT[:, c0:c0 + 2],
                                sb[:, 128 * k2:128 * (k2 + 1)], ident2[:])

    nc.vector.tensor_copy(lhsT[:], spanT[:])

    # ---- final projection: ans[j=(b,t'), d] = sum_k spanT[k, j] * W[k, d]
    ap = ans_ps.tile([2 * BPC, D], F32)
    for k2 in range(2):
        nc.tensor.matmul(
            ap[:],
            lhsT[:, 16 * k2:16 * (k2 + 1)],
            w_sb[:, k2, :],
            start=(k2 == 0),
            stop=(k2 == 1),
        )
    ans_sb = small.tile([2 * BPC, D], F32)
    nc.vector.tensor_copy(ans_sb[:], ap[:])
    nc.scalar.dma_start(out, ans_sb[:])


def _build():
    nc = bacc.Bacc("TRN2", target_bir_lowering=False, debug=False,
                   num_devices=N_CORES)
    doc = nc.dram_tensor("doc", [BPC, S, D], F32R, kind="ExternalInput").ap()
    scores = nc.dram_tensor("scores", [2, BPC, S], F32, kind="ExternalInput").ap()
    gold = nc.dram_tensor("gold", [2, BPC], F32, kind="ExternalInput").ap()
    w = nc.dram_tensor("w", [D, D], F32R, kind="ExternalInput").ap()
    out = nc.dram_tensor("out", [2 * BPC, D], F32, kind="ExternalOutput").ap()
    idx_out = nc.dram_tensor("idx_out", [2 * BPC, 1], F32, kind="ExternalOutput").ap()

    with tile.TileContext(nc) as tc, ExitStack() as ctx:
        _emit(ctx, tc, doc, scores, gold, w, out, idx_out)
    nc.compile()
    return nc


def get_nc():
    if "nc" not in _cache:
        _cache["nc"] = _build()
    return _cache["nc"]


def make_in_maps(doc_encoding, score_soft, answer_idx, W_mlp):
    doc_encoding = np.asarray(doc_encoding, dtype=np.float32)
    score_soft = np.asarray(score_soft, dtype=np.float32)
    gold_f = np.asarray(answer_idx).astype(np.float32)
    w = np.ascontiguousarray(np.asarray(W_mlp, dtype=np.float32))
    maps = []
    for c in range(N_CORES):
        sl = slice(c * BPC, (c + 1) * BPC)
        maps.append({
            "doc": np.ascontiguousarray(doc_encoding[sl]),
            "scores": np.ascontiguousarray(score_soft[:, sl]),
            "gold": np.ascontiguousarray(gold_f[:, sl]),
            "w": w,
        })
    return maps


def assemble(results, answer_idx, b_mlp):
    """results: list of 8 per-core dicts with 'out' [16, D] and 'idx_out' [16, 1]."""
    b_mlp = np.asarray(b_mlp, dtype=np.float32)
    ans = np.empty((B, D), np.float32)
    goldo = np.empty((B, D), np.float32)
    for c in range(N_CORES):
        sl = slice(c * BPC, (c + 1) * BPC)
        o = np.asarray(results[c]["out"]).reshape(BPC, 2, D)
        ans[sl] = o[:, 0]
        goldo[sl] = o[:, 1]
        if b_mlp.any():
            idx = np.asarray(results[c]["idx_out"]).reshape(2 * BPC)
            cnt_h = np.maximum(0.0, idx[BPC:] - idx[:BPC] + 1.0)
            gi = np.asarray(answer_idx)[:, sl].astype(np.float64)
            cnt_g = np.maximum(0.0, gi[1] - gi[0] + 1.0)
            ans[sl] += cnt_h[:, None].astype(np.float32) * b_mlp[None, :]
            goldo[sl] += cnt_g[:, None].astype(np.float32) * b_mlp[None, :]
    return ans, goldo


def kernel(doc_encoding, score_soft, answer_idx, W_mlp, b_mlp):
    nc = get_nc()
    in_maps = make_in_maps(doc_encoding, score_soft, answer_idx, W_mlp)
    res = run_bass_kernel_spmd(nc, in_maps, list(range(N_CORES))).results
    return assemble(res, answer_idx, b_mlp)


# revision 7
# speedup vs baseline: 1.3072x; 1.0039x over previous
"""Trainium2 Bass kernel for nn_Discriminator_44779329028358 (segment_reduce).

Math (per batch b):
    doc_proj = doc_encoding[b] @ W + bias            # [S, D]
    hard span: s_h = argmax(score_soft[0,b]), e_h = argmax(score_soft[1,b])
    gold span: (s_g, e_g) = answer_idx[:, b]
    answer[b]      = sum_{s_h<=s<=e_h} doc_proj[s]   # zeros when s_h > e_h
    answer_gold[b] = sum_{s_g<=s<=e_g} doc_proj[s]

Kernel strategy: masked span-sum over raw doc_encoding first (matmul with a
0/1 mask as the stationary operand, streaming doc tiles), then a single tiny
projection through W:  (mask @ doc) @ W == mask @ (doc @ W).  The count*bias
term is added on the host from the (returned) span indices; bias is zeros for
this problem's inputs anyway.

Sharding: data-parallel over batch across 8 cores (8 batches/core).
"""

from contextlib import ExitStack

import numpy as np

import concourse.bacc as bacc
import concourse.mybir as mybir
import concourse.tile as tile
from concourse.bass_utils import run_bass_kernel_spmd
from concourse.masks import make_identity

N_CORES = 8
B, S, D = 64, 2048, 256
BPC = B // N_CORES          # batches per core = 8
KT = S // 128               # k-tiles per batch = 16
F32 = mybir.dt.float32
F32R = mybir.dt.float32r

_cache = {}


def _emit(ctx, tc, doc, scores, gold, w, out, idx_out):
    nc = tc.nc

    const = ctx.enter_context(tc.tile_pool(name="const", bufs=1))
    small = ctx.enter_context(tc.tile_pool(name="small", bufs=1))
    docp = ctx.enter_context(tc.tile_pool(name="doc", bufs=4))
    span_ps = ctx.enter_context(tc.tile_pool(name="span_ps", bufs=2, space="PSUM"))
    aux_ps = ctx.enter_context(tc.tile_pool(name="aux_ps", bufs=1, space="PSUM"))
    ans_ps = ctx.enter_context(tc.tile_pool(name="ans_ps", bufs=1, space="PSUM"))

    # All small DMAs ride the scalar (ACT) HWDGE ring so they never queue
    # behind the 16 MiB doc stream on the sync (SP) ring; no gpsimd (SWDGE)
    # DMAs anywhere so the broadcast path never waits on a Q7 drain.

    # ---- scores + argmax (critical path for the masks) ----
    # 16 problems on 16 partitions: p = t*BPC + b  (t: 0=start scores, 1=end)
    sc = small.tile([2 * BPC, S], F32)
    nc.scalar.dma_start(sc[:], scores.rearrange("t b s -> (t b) s"))

    # ---- index row, free-axis layout [1, 32]: cols 0:8 hs, 8:16 gs, 16:24 he, 24:32 ge
    idx_row = small.tile([1, 32], F32)
    nc.scalar.dma_start(idx_row[0:1, 8:8 + BPC], gold[0:1, :])          # gold s
    nc.scalar.dma_start(idx_row[0:1, 24:24 + BPC], gold[1:2, :])        # gold e

    # ---- constants ----
    w_sb = const.tile([128, 2, D], F32R)          # W rows (k2*128+p) -> [p, k2, d]
    nc.scalar.dma_start(w_sb[:], w.rearrange("(k2 p) d -> p k2 d", p=128))

    ident2 = const.tile([2, 2], F32)
    make_identity(nc, ident2[:])
    ident16 = const.tile([16, 16], F32)
    make_identity(nc, ident16[:])
    ones1 = const.tile([1, 128], F32)
    nc.gpsimd.memset(ones1[:], 1.0)

    iota_i = const.tile([128, KT], mybir.dt.int32)   # val[p, kt] = p*KT + kt
    nc.gpsimd.iota(iota_i[:], pattern=[[1, KT]], base=0, channel_multiplier=KT)
    iota_f = const.tile([128, KT], F32)
    nc.vector.tensor_copy(iota_f[:], iota_i[:])

    mx = small.tile([2 * BPC, 8], F32)
    mi = small.tile([2 * BPC, 8], mybir.dt.uint32)
    nc.vector.max(mx[:], sc[:])
    nc.vector.max_index(mi[:], mx[:], sc[:])
    idxf = small.tile([2 * BPC, 1], F32)
    nc.vector.tensor_copy(idxf[:], mi[:, 0:1])       # uint32 -> f32 convert
    nc.scalar.dma_start(idx_out, idxf[:])

    # hard indices partition->free via PE transpose: [16,1] -> [1,16]
    idxT_ps = aux_ps.tile([1, 16], F32, tag="idxT")
    nc.tensor.transpose(idxT_ps[:], idxf[:], ident16[:])
    # psum cols 0:8 = hard s -> idx_row 0:8; cols 8:16 = hard e -> idx_row 16:24
    nc.vector.tensor_copy(
        idx_row[0:1, 0:32].rearrange("o (t x) -> o t x", t=2)[:, :, 0:BPC],
        idxT_ps[0:1, :].rearrange("o (t b) -> o t b", t=2),
    )

    # broadcast idx_row across partitions on the (idle) PE — a gpsimd
    # partition_broadcast would force an 11us SWDGE drain behind the doc DMAs
    bc_ps = aux_ps.tile([128, 32], F32, tag="bcast")
    nc.tensor.matmul(bc_ps[:], ones1[:], idx_row[:], start=True, stop=True)
    idx_bc = small.tile([128, 32], F32)
    nc.vector.tensor_copy(idx_bc[:], bc_ps[:])

    # ---- masks: mask[p, kt, b, t'] = (pos >= s) & (pos <= e), pos = p*KT + kt
    # column order per (kt): j = 2*b + t'  (t'=0 hard, 1 gold)
    s_view = idx_bc[:, 0:16].rearrange("p (t b) -> p t b", t=2).transpose([0, 2, 1])
    e_view = idx_bc[:, 16:32].rearrange("p (t b) -> p t b", t=2).transpose([0, 2, 1])
    iota_b = iota_f[:, :, None, None].broadcast_to([128, KT, BPC, 2])

    ge = small.tile([128, KT, BPC, 2], F32)
    le = small.tile([128, KT, BPC, 2], F32)
    mask = small.tile([128, KT, BPC, 2], F32R)
    nc.vector.tensor_tensor(ge[:], iota_b, s_view[:, None, :, :].broadcast_to([128, KT, BPC, 2]), mybir.AluOpType.is_ge)
    nc.vector.tensor_tensor(le[:], iota_b, e_view[:, None, :, :].broadcast_to([128, KT, BPC, 2]), mybir.AluOpType.is_le)
    nc.vector.tensor_tensor(mask[:], ge[:], le[:], mybir.AluOpType.mult)

    # ---- per-batch span sums + transposes ----
    # spanT collects transposed span sums: col 4*b + 2*k2 + t'
    spanT = aux_ps.tile([128, 4 * BPC], F32)
    lhsT = small.tile([128, 4 * BPC], F32R)

    for b in range(BPC):
        dt_ = docp.tile([128, KT, D], F32R)
        # doc rows r = p*KT + kt; contiguous 16 KiB per partition
        nc.sync.dma_start(dt_[:], doc[b].rearrange("(p x) d -> p x d", p=128))

        ps = span_ps.tile([2, D], F32)
        for kt in range(KT):
            nc.tensor.matmul(
                ps[:],
                mask[:, kt, b, :],
                dt_[:, kt, :],
                start=(kt == 0),
                stop=(kt == KT - 1),
            )
        sb = small.tile([2, D], F32, tag="span_sb")
        nc.vector.tensor_copy(sb[:], ps[:])
        # transpose [2, 128-half] -> [128, 2] into spanT columns
        # layout: col = 16*k2 + 2*b + t'  (k-half-major, contiguous per half)
        for k2 in range(2):
            c0 = 16 * k2 + 2 * b
            nc.tensor.transpose(spanT[:, c0:c0 + 2],
                                sb[:, 128 * k2:128 * (k2 + 1)], ident2[:])

    nc.vector.tensor_copy(lhsT[:], spanT[:])

    # ---- final projection: ans[j=(b,t'), d] = sum_k spanT[k, j] * W[k, d]
    ap = ans_ps.tile([2 * BPC, D], F32)
    for k2 in range(2):
        nc.tensor.matmul(
            ap[:],
            lhsT[:, 16 * k2:16 * (k2 + 1)],
            w_sb[:, k2, :],
            start=(k2 == 0),
            stop=(k2 == 1),
        )
    ans_sb = small.tile([2 * BPC, D], F32)
    nc.vector.tensor_copy(ans_sb[:], ap[:])
    nc.scalar.dma_start(out, ans_sb[:])


def _build():
    nc = bacc.Bacc("TRN2", target_bir_lowering=False, debug=False,
                   num_devices=N_CORES)
    doc = nc.dram_tensor("doc", [BPC, S, D], F32R, kind="ExternalInput").ap()
    scores = nc.dram_tensor("scores", [2, BPC, S], F32, kind="ExternalInput").ap()
    gold = nc.dram_tensor("gold", [2, BPC], F32, kind="ExternalInput").ap()
    w = nc.dram_tensor("w", [D, D], F32R, kind="ExternalInput").ap()
    out = nc.dram_tensor("out", [2 * BPC, D], F32, kind="ExternalOutput").ap()
    idx_out = nc.dram_tensor("idx_out", [2 * BPC, 1], F32, kind="ExternalOutput").ap()

    with tile.TileContext(nc) as tc, ExitStack() as ctx:
        _emit(ctx, tc, doc, scores, gold, w, out, idx_out)
    nc.compile()
    return nc


def get_nc():
    if "nc" not in _cache:
        _cache["nc"] = _build()
    return _cache["nc"]


def make_in_maps(doc_encoding, score_soft, answer_idx, W_mlp):
    doc_encoding = np.asarray(doc_encoding, dtype=np.float32)
    score_soft = np.asarray(score_soft, dtype=np.float32)
    gold_f = np.asarray(answer_idx).astype(np.float32)
    w = np.ascontiguousarray(np.asarray(W_mlp, dtype=np.float32))
    maps = []
    for c in range(N_CORES):
        sl = slice(c * BPC, (c + 1) * BPC)
        maps.append({
            "doc": np.ascontiguousarray(doc_encoding[sl]),
            "scores": np.ascontiguousarray(score_soft[:, sl]),
            "gold": np.ascontiguousarray(gold_f[:, sl]),
            "w": w,
        })
    return maps


def assemble(results, answer_idx, b_mlp):
    """results: list of 8 per-core dicts with 'out' [16, D] and 'idx_out' [16, 1]."""
    b_mlp = np.asarray(b_mlp, dtype=np.float32)
    ans = np.empty((B, D), np.float32)
    goldo = np.empty((B, D), np.float32)
    for c in range(N_CORES):
        sl = slice(c * BPC, (c + 1) * BPC)
        o = np.asarray(results[c]["out"]).reshape(BPC, 2, D)
        ans[sl] = o[:, 0]
        goldo[sl] = o[:, 1]
        if b_mlp.any():
            idx = np.asarray(results[c]["idx_out"]).reshape(2 * BPC)
            cnt_h = np.maximum(0.0, idx[BPC:] - idx[:BPC] + 1.0)
            gi = np.asarray(answer_idx)[:, sl].astype(np.float64)
            cnt_g = np.maximum(0.0, gi[1] - gi[0] + 1.0)
            ans[sl] += cnt_h[:, None].astype(np.float32) * b_mlp[None, :]
            goldo[sl] += cnt_g[:, None].astype(np.float32) * b_mlp[None, :]
    return ans, goldo


def kernel(doc_encoding, score_soft, answer_idx, W_mlp, b_mlp):
    nc = get_nc()
    in_maps = make_in_maps(doc_encoding, score_soft, answer_idx, W_mlp)
    res = run_bass_kernel_spmd(nc, in_maps, list(range(N_CORES))).results
    return assemble(res, answer_idx, b_mlp)


# revision 8
# speedup vs baseline: 1.3922x; 1.0651x over previous
"""Trainium2 Bass kernel for nn_Discriminator_44779329028358 (segment_reduce).

Math (per batch b):
    doc_proj = doc_encoding[b] @ W + bias            # [S, D]
    hard span: s_h = argmax(score_soft[0,b]), e_h = argmax(score_soft[1,b])
    gold span: (s_g, e_g) = answer_idx[:, b]
    answer[b]      = sum_{s_h<=s<=e_h} doc_proj[s]   # zeros when s_h > e_h
    answer_gold[b] = sum_{s_g<=s<=e_g} doc_proj[s]

Kernel strategy: masked span-sum over raw doc_encoding first (matmul with a
0/1 mask as the stationary operand, streaming doc tiles), then a single tiny
projection through W:  (mask @ doc) @ W == mask @ (doc @ W).  The count*bias
term is added on the host from the (returned) span indices; bias is zeros for
this problem's inputs anyway.

Sharding: data-parallel over batch across 8 cores (8 batches/core).
"""

from contextlib import ExitStack

import numpy as np

import concourse.bacc as bacc
import concourse.mybir as mybir
import concourse.tile as tile
from concourse.bass_utils import run_bass_kernel_spmd
from concourse.masks import make_identity

N_CORES = 8
B, S, D = 64, 2048, 256
BPC = B // N_CORES          # batches per core = 8
KT = S // 128               # k-tiles per batch = 16
F32 = mybir.dt.float32
F32R = mybir.dt.float32r

_cache = {}


def _emit(ctx, tc, doc, scores, gold, w, out, idx_out):
    nc = tc.nc

    const = ctx.enter_context(tc.tile_pool(name="const", bufs=1))
    small = ctx.enter_context(tc.tile_pool(name="small", bufs=1))
    docp = ctx.enter_context(tc.tile_pool(name="doc", bufs=4))
    span_ps = ctx.enter_context(tc.tile_pool(name="span_ps", bufs=2, space="PSUM"))
    aux_ps = ctx.enter_context(tc.tile_pool(name="aux_ps", bufs=1, space="PSUM"))
    ans_ps = ctx.enter_context(tc.tile_pool(name="ans_ps", bufs=1, space="PSUM"))

    # All small DMAs ride the scalar (ACT) HWDGE ring so they never queue
    # behind the 16 MiB doc stream on the sync (SP) ring; no gpsimd (SWDGE)
    # DMAs anywhere so the broadcast path never waits on a Q7 drain.

    # ---- scores + argmax (critical path for the masks) ----
    # 16 problems on 16 partitions: p = t*BPC + b  (t: 0=start scores, 1=end)
    sc = small.tile([2 * BPC, S], F32)
    nc.scalar.dma_start(sc[:], scores.rearrange("t b s -> (t b) s"))

    # ---- index row, free-axis layout [1, 32]: cols 0:8 hs, 8:16 gs, 16:24 he, 24:32 ge
    idx_row = small.tile([1, 32], F32)
    nc.scalar.dma_start(idx_row[0:1, 8:8 + BPC], gold[0:1, :])          # gold s
    nc.scalar.dma_start(idx_row[0:1, 24:24 + BPC], gold[1:2, :])        # gold e

    # ---- constants ----
    w_sb = const.tile([128, 2, D], F32R)          # W rows (k2*128+p) -> [p, k2, d]
    nc.scalar.dma_start(w_sb[:], w.rearrange("(k2 p) d -> p k2 d", p=128))

    ident2 = const.tile([2, 2], F32)
    make_identity(nc, ident2[:])
    ident16 = const.tile([16, 16], F32)
    make_identity(nc, ident16[:])
    ones1 = const.tile([1, 128], F32)
    nc.gpsimd.memset(ones1[:], 1.0)

    iota_i = const.tile([128, KT], mybir.dt.int32)   # val[p, kt] = p*KT + kt
    nc.gpsimd.iota(iota_i[:], pattern=[[1, KT]], base=0, channel_multiplier=KT)
    iota_f = const.tile([128, KT], F32)
    nc.vector.tensor_copy(iota_f[:], iota_i[:])

    mx = small.tile([2 * BPC, 8], F32)
    mi = small.tile([2 * BPC, 8], mybir.dt.uint32)
    nc.vector.max(mx[:], sc[:])
    nc.vector.max_index(mi[:], mx[:], sc[:])
    idxf = small.tile([2 * BPC, 1], F32)
    nc.vector.tensor_copy(idxf[:], mi[:, 0:1])       # uint32 -> f32 convert
    nc.scalar.dma_start(idx_out, idxf[:])

    # hard indices partition->free via PE transpose: [16,1] -> [1,16]
    idxT_ps = aux_ps.tile([1, 16], F32, tag="idxT")
    nc.tensor.transpose(idxT_ps[:], idxf[:], ident16[:])
    # psum cols 0:8 = hard s -> idx_row 0:8; cols 8:16 = hard e -> idx_row 16:24
    nc.vector.tensor_copy(
        idx_row[0:1, 0:32].rearrange("o (t x) -> o t x", t=2)[:, :, 0:BPC],
        idxT_ps[0:1, :].rearrange("o (t b) -> o t b", t=2),
    )

    # broadcast idx_row across partitions on the (idle) PE — a gpsimd
    # partition_broadcast would force an 11us SWDGE drain behind the doc DMAs
    bc_ps = aux_ps.tile([128, 32], F32, tag="bcast")
    nc.tensor.matmul(bc_ps[:], ones1[:], idx_row[:], start=True, stop=True)
    idx_bc = small.tile([128, 32], F32)
    nc.vector.tensor_copy(idx_bc[:], bc_ps[:])

    # ---- masks: mask[p, kt, b, t'] = (pos >= s) & (pos <= e), pos = p*KT + kt
    # column order per (kt): j = 2*b + t'  (t'=0 hard, 1 gold)
    s_view = idx_bc[:, 0:16].rearrange("p (t b) -> p t b", t=2).transpose([0, 2, 1])
    e_view = idx_bc[:, 16:32].rearrange("p (t b) -> p t b", t=2).transpose([0, 2, 1])
    iota_b = iota_f[:, :, None, None].broadcast_to([128, KT, BPC, 2])

    ge = small.tile([128, KT, BPC, 2], F32)
    le = small.tile([128, KT, BPC, 2], F32)
    mask = small.tile([128, KT, BPC, 2], F32R)
    nc.vector.tensor_tensor(ge[:], iota_b, s_view[:, None, :, :].broadcast_to([128, KT, BPC, 2]), mybir.AluOpType.is_ge)
    nc.vector.tensor_tensor(le[:], iota_b, e_view[:, None, :, :].broadcast_to([128, KT, BPC, 2]), mybir.AluOpType.is_le)
    nc.vector.tensor_tensor(mask[:], ge[:], le[:], mybir.AluOpType.mult)

    # ---- per-batch span sums + transposes ----
    # spanT collects transposed span sums: col 4*b + 2*k2 + t'
    spanT = aux_ps.tile([128, 4 * BPC], F32)
    lhsT = small.tile([128, 4 * BPC], F32R)

    HALF = KT // 2
    for b in range(BPC):
        # doc rows r = p*KT + kt; two half-batch tiles (1 MiB each) so PE can
        # start on the first half sooner and idle gaps stay under the HAM
        # re-throttle window
        dsrc = doc[b].rearrange("(p x) d -> p x d", p=128)
        dtA = docp.tile([128, HALF, D], F32R, tag="dA")
        nc.sync.dma_start(dtA[:], dsrc[:, 0:HALF, :])
        dtB = docp.tile([128, HALF, D], F32R, tag="dB")
        nc.sync.dma_start(dtB[:], dsrc[:, HALF:KT, :])

        ps = span_ps.tile([2, D], F32)
        for kt in range(KT):
            src = dtA if kt < HALF else dtB
            nc.tensor.matmul(
                ps[:],
                mask[:, kt, b, :],
                src[:, kt % HALF, :],
                start=(kt == 0),
                stop=(kt == KT - 1),
            )
        sb = small.tile([2, D], F32, tag="span_sb")
        nc.vector.tensor_copy(sb[:], ps[:])
        # transpose [2, 128-half] -> [128, 2] into spanT columns
        # layout: col = 16*k2 + 2*b + t'  (k-half-major, contiguous per half)
        for k2 in range(2):
            c0 = 16 * k2 + 2 * b
            nc.tensor.transpose(spanT[:, c0:c0 + 2],
                                sb[:, 128 * k2:128 * (k2 + 1)], ident2[:])

    nc.vector.tensor_copy(lhsT[:], spanT[:])

    # ---- final projection: ans[j=(b,t'), d] = sum_k spanT[k, j] * W[k, d]
    ap = ans_ps.tile([2 * BPC, D], F32)
    for k2 in range(2):
        nc.tensor.matmul(
            ap[:],
            lhsT[:, 16 * k2:16 * (k2 + 1)],
            w_sb[:, k2, :],
            start=(k2 == 0),
            stop=(k2 == 1),
        )
    ans_sb = small.tile([2 * BPC, D], F32)
    nc.vector.tensor_copy(ans_sb[:], ap[:])
    nc.scalar.dma_start(out, ans_sb[:])


def _build():
    nc = bacc.Bacc("TRN2", target_bir_lowering=False, debug=False,
                   num_devices=N_CORES)
    doc = nc.dram_tensor("doc", [BPC, S, D], F32R, kind="ExternalInput").ap()
    scores = nc.dram_tensor("scores", [2, BPC, S], F32, kind="ExternalInput").ap()
    gold = nc.dram_tensor("gold", [2, BPC], F32, kind="ExternalInput").ap()
    w = nc.dram_tensor("w", [D, D], F32R, kind="ExternalInput").ap()
    out = nc.dram_tensor("out", [2 * BPC, D], F32, kind="ExternalOutput").ap()
    idx_out = nc.dram_tensor("idx_out", [2 * BPC, 1], F32, kind="ExternalOutput").ap()

    with tile.TileContext(nc) as tc, ExitStack() as ctx:
        _emit(ctx, tc, doc, scores, gold, w, out, idx_out)
    nc.compile()
    return nc


def get_nc():
    if "nc" not in _cache:
        _cache["nc"] = _build()
    return _cache["nc"]


def make_in_maps(doc_encoding, score_soft, answer_idx, W_mlp):
    doc_encoding = np.asarray(doc_encoding, dtype=np.float32)
    score_soft = np.asarray(score_soft, dtype=np.float32)
    gold_f = np.asarray(answer_idx).astype(np.float32)
    w = np.ascontiguousarray(np.asarray(W_mlp, dtype=np.float32))
    maps = []
    for c in range(N_CORES):
        sl = slice(c * BPC, (c + 1) * BPC)
        maps.append({
            "doc": np.ascontiguousarray(doc_encoding[sl]),
            "scores": np.ascontiguousarray(score_soft[:, sl]),
            "gold": np.ascontiguousarray(gold_f[:, sl]),
            "w": w,
        })
    return maps


def assemble(results, answer_idx, b_mlp):
    """results: list of 8 per-core dicts with 'out' [16, D] and 'idx_out' [16, 1]."""
    b_mlp = np.asarray(b_mlp, dtype=np.float32)
    ans = np.empty((B, D), np.float32)
    goldo = np.empty((B, D), np.float32)
    for c in range(N_CORES):
        sl = slice(c * BPC, (c + 1) * BPC)
        o = np.asarray(results[c]["out"]).reshape(BPC, 2, D)
        ans[sl] = o[:, 0]
        goldo[sl] = o[:, 1]
        if b_mlp.any():
            idx = np.asarray(results[c]["idx_out"]).reshape(2 * BPC)
            cnt_h = np.maximum(0.0, idx[BPC:] - idx[:BPC] + 1.0)
            gi = np.asarray(answer_idx)[:, sl].astype(np.float64)
            cnt_g = np.maximum(0.0, gi[1] - gi[0] + 1.0)
            ans[sl] += cnt_h[:, None].astype(np.float32) * b_mlp[None, :]
            goldo[sl] += cnt_g[:, None].astype(np.float32) * b_mlp[None, :]
    return ans, goldo


def kernel(doc_encoding, score_soft, answer_idx, W_mlp, b_mlp):
    nc = get_nc()
    in_maps = make_in_maps(doc_encoding, score_soft, answer_idx, W_mlp)
    res = run_bass_kernel_spmd(nc, in_maps, list(range(N_CORES))).results
    return assemble(res, answer_idx, b_mlp)


# revision 9
# speedup vs baseline: 1.5368x; 1.1039x over previous
"""Trainium2 Bass kernel for nn_Discriminator_44779329028358 (segment_reduce).

Math (per batch b):
    doc_proj = doc_encoding[b] @ W + bias            # [S, D]
    hard span: s_h = argmax(score_soft[0,b]), e_h = argmax(score_soft[1,b])
    gold span: (s_g, e_g) = answer_idx[:, b]
    answer[b]      = sum_{s_h<=s<=e_h} doc_proj[s]   # zeros when s_h > e_h
    answer_gold[b] = sum_{s_g<=s<=e_g} doc_proj[s]

Kernel strategy: masked span-sum over raw doc_encoding first (matmul with a
0/1 mask as the stationary operand, streaming doc tiles), then a single tiny
projection through W:  (mask @ doc) @ W == mask @ (doc @ W).  The count*bias
term is added on the host from the (returned) span indices; bias is zeros for
this problem's inputs anyway.

Sharding: data-parallel over batch across 8 cores (8 batches/core).
"""

from contextlib import ExitStack

import numpy as np

import concourse.bacc as bacc
import concourse.mybir as mybir
import concourse.tile as tile
from concourse.bass_utils import run_bass_kernel_spmd
from concourse.masks import make_identity

N_CORES = 8
B, S, D = 64, 2048, 256
BPC = B // N_CORES          # batches per core = 8
KT = S // 128               # k-tiles per batch = 16
F32 = mybir.dt.float32
F32R = mybir.dt.float32r

_cache = {}


def _emit(ctx, tc, doc, scores, gold, w, out, idx_out):
    nc = tc.nc

    const = ctx.enter_context(tc.tile_pool(name="const", bufs=1))
    small = ctx.enter_context(tc.tile_pool(name="small", bufs=1))
    docp = ctx.enter_context(tc.tile_pool(name="doc", bufs=10))
    span_ps = ctx.enter_context(tc.tile_pool(name="span_ps", bufs=2, space="PSUM"))
    aux_ps = ctx.enter_context(tc.tile_pool(name="aux_ps", bufs=1, space="PSUM"))
    ans_ps = ctx.enter_context(tc.tile_pool(name="ans_ps", bufs=1, space="PSUM"))

    # All small DMAs ride the scalar (ACT) HWDGE ring so they never queue
    # behind the 16 MiB doc stream on the sync (SP) ring; no gpsimd (SWDGE)
    # DMAs anywhere so the broadcast path never waits on a Q7 drain.

    # ---- scores + argmax (critical path for the masks) ----
    # 16 problems on 16 partitions: p = t*BPC + b  (t: 0=start scores, 1=end)
    sc = small.tile([2 * BPC, S], F32)
    nc.scalar.dma_start(sc[:], scores.rearrange("t b s -> (t b) s"))

    # ---- index row, free-axis layout [1, 32]: cols 0:8 hs, 8:16 gs, 16:24 he, 24:32 ge
    idx_row = small.tile([1, 32], F32)
    nc.scalar.dma_start(idx_row[0:1, 8:8 + BPC], gold[0:1, :])          # gold s
    nc.scalar.dma_start(idx_row[0:1, 24:24 + BPC], gold[1:2, :])        # gold e

    # ---- constants ----
    w_sb = const.tile([128, 2, D], F32R)          # W rows (k2*128+p) -> [p, k2, d]
    nc.scalar.dma_start(w_sb[:], w.rearrange("(k2 p) d -> p k2 d", p=128))

    ident2 = const.tile([2, 2], F32)
    make_identity(nc, ident2[:])
    ident16 = const.tile([16, 16], F32)
    make_identity(nc, ident16[:])
    ones1 = const.tile([1, 128], F32)
    nc.gpsimd.memset(ones1[:], 1.0)

    iota_i = const.tile([128, KT], mybir.dt.int32)   # val[p, kt] = p*KT + kt
    nc.gpsimd.iota(iota_i[:], pattern=[[1, KT]], base=0, channel_multiplier=KT)
    iota_f = const.tile([128, KT], F32)
    nc.vector.tensor_copy(iota_f[:], iota_i[:])

    mx = small.tile([2 * BPC, 8], F32)
    mi = small.tile([2 * BPC, 8], mybir.dt.uint32)
    nc.vector.max(mx[:], sc[:])
    nc.vector.max_index(mi[:], mx[:], sc[:])
    idxf = small.tile([2 * BPC, 1], F32)
    nc.vector.tensor_copy(idxf[:], mi[:, 0:1])       # uint32 -> f32 convert
    nc.scalar.dma_start(idx_out, idxf[:])

    # hard indices partition->free via PE transpose: [16,1] -> [1,16]
    idxT_ps = aux_ps.tile([1, 16], F32, tag="idxT")
    nc.tensor.transpose(idxT_ps[:], idxf[:], ident16[:])
    # psum cols 0:8 = hard s -> idx_row 0:8; cols 8:16 = hard e -> idx_row 16:24
    nc.vector.tensor_copy(
        idx_row[0:1, 0:32].rearrange("o (t x) -> o t x", t=2)[:, :, 0:BPC],
        idxT_ps[0:1, :].rearrange("o (t b) -> o t b", t=2),
    )

    # broadcast idx_row across partitions on the (idle) PE — a gpsimd
    # partition_broadcast would force an 11us SWDGE drain behind the doc DMAs
    bc_ps = aux_ps.tile([128, 32], F32, tag="bcast")
    nc.tensor.matmul(bc_ps[:], ones1[:], idx_row[:], start=True, stop=True)
    idx_bc = small.tile([128, 32], F32)
    nc.vector.tensor_copy(idx_bc[:], bc_ps[:])

    # ---- masks: mask[p, kt, b, t'] = (pos >= s) & (pos <= e), pos = p*KT + kt
    # column order per (kt): j = 2*b + t'  (t'=0 hard, 1 gold)
    s_view = idx_bc[:, 0:16].rearrange("p (t b) -> p t b", t=2).transpose([0, 2, 1])
    e_view = idx_bc[:, 16:32].rearrange("p (t b) -> p t b", t=2).transpose([0, 2, 1])
    iota_b = iota_f[:, :, None, None].broadcast_to([128, KT, BPC, 2])

    ge = small.tile([128, KT, BPC, 2], F32)
    le = small.tile([128, KT, BPC, 2], F32)
    mask = small.tile([128, KT, BPC, 2], F32R)
    nc.vector.tensor_tensor(ge[:], iota_b, s_view[:, None, :, :].broadcast_to([128, KT, BPC, 2]), mybir.AluOpType.is_ge)
    nc.vector.tensor_tensor(le[:], iota_b, e_view[:, None, :, :].broadcast_to([128, KT, BPC, 2]), mybir.AluOpType.is_le)
    nc.vector.tensor_tensor(mask[:], ge[:], le[:], mybir.AluOpType.mult)

    # ---- per-batch span sums + transposes ----
    # spanT collects transposed span sums: col 4*b + 2*k2 + t'
    spanT = aux_ps.tile([128, 4 * BPC], F32)
    lhsT = small.tile([128, 4 * BPC], F32R)

    HALF = KT // 2
    for b in range(BPC):
        # doc rows r = p*KT + kt; two half-batch tiles (1 MiB each) so PE can
        # start on the first half sooner and idle gaps stay under the HAM
        # re-throttle window
        dsrc = doc[b].rearrange("(p x) d -> p x d", p=128)
        dtA = docp.tile([128, HALF, D], F32R, tag="dA")
        nc.sync.dma_start(dtA[:], dsrc[:, 0:HALF, :])
        dtB = docp.tile([128, HALF, D], F32R, tag="dB")
        nc.sync.dma_start(dtB[:], dsrc[:, HALF:KT, :])

        ps = span_ps.tile([2, D], F32)
        for kt in range(KT):
            src = dtA if kt < HALF else dtB
            nc.tensor.matmul(
                ps[:],
                mask[:, kt, b, :],
                src[:, kt % HALF, :],
                start=(kt == 0),
                stop=(kt == KT - 1),
            )
        sb = small.tile([2, D], F32, tag="span_sb")
        nc.vector.tensor_copy(sb[:], ps[:])
        # transpose [2, 128-half] -> [128, 2] into spanT columns
        # layout: col = 16*k2 + 2*b + t'  (k-half-major, contiguous per half)
        for k2 in range(2):
            c0 = 16 * k2 + 2 * b
            nc.tensor.transpose(spanT[:, c0:c0 + 2],
                                sb[:, 128 * k2:128 * (k2 + 1)], ident2[:])

    nc.vector.tensor_copy(lhsT[:], spanT[:])

    # ---- final projection: ans[j=(b,t'), d] = sum_k spanT[k, j] * W[k, d]
    ap = ans_ps.tile([2 * BPC, D], F32)
    for k2 in range(2):
        nc.tensor.matmul(
            ap[:],
            lhsT[:, 16 * k2:16 * (k2 + 1)],
            w_sb[:, k2, :],
            start=(k2 == 0),
            stop=(k2 == 1),
        )
    ans_sb = small.tile([2 * BPC, D], F32)
    nc.vector.tensor_copy(ans_sb[:], ap[:])
    nc.scalar.dma_start(out, ans_sb[:])


def _build():
    nc = bacc.Bacc("TRN2", target_bir_lowering=False, debug=False,
                   num_devices=N_CORES)
    doc = nc.dram_tensor("doc", [BPC, S, D], F32R, kind="ExternalInput").ap()
    scores = nc.dram_tensor("scores", [2, BPC, S], F32, kind="ExternalInput").ap()
    gold = nc.dram_tensor("gold", [2, BPC], F32, kind="ExternalInput").ap()
    w = nc.dram_tensor("w", [D, D], F32R, kind="ExternalInput").ap()
    out = nc.dram_tensor("out", [2 * BPC, D], F32, kind="ExternalOutput").ap()
    idx_out = nc.dram_tensor("idx_out", [2 * BPC, 1], F32, kind="ExternalOutput").ap()

    with tile.TileContext(nc) as tc, ExitStack() as ctx:
        _emit(ctx, tc, doc, scores, gold, w, out, idx_out)
    nc.compile()
    return nc


def get_nc():
    if "nc" not in _cache:
        _cache["nc"] = _build()
    return _cache["nc"]


def make_in_maps(doc_encoding, score_soft, answer_idx, W_mlp):
    doc_encoding = np.asarray(doc_encoding, dtype=np.float32)
    score_soft = np.asarray(score_soft, dtype=np.float32)
    gold_f = np.asarray(answer_idx).astype(np.float32)
    w = np.ascontiguousarray(np.asarray(W_mlp, dtype=np.float32))
    maps = []
    for c in range(N_CORES):
        sl = slice(c * BPC, (c + 1) * BPC)
        maps.append({
            "doc": np.ascontiguousarray(doc_encoding[sl]),
            "scores": np.ascontiguousarray(score_soft[:, sl]),
            "gold": np.ascontiguousarray(gold_f[:, sl]),
            "w": w,
        })
    return maps


def assemble(results, answer_idx, b_mlp):
    """results: list of 8 per-core dicts with 'out' [16, D] and 'idx_out' [16, 1]."""
    b_mlp = np.asarray(b_mlp, dtype=np.float32)
    ans = np.empty((B, D), np.float32)
    goldo = np.empty((B, D), np.float32)
    for c in range(N_CORES):
        sl = slice(c * BPC, (c + 1) * BPC)
        o = np.asarray(results[c]["out"]).reshape(BPC, 2, D)
        ans[sl] = o[:, 0]
        goldo[sl] = o[:, 1]
        if b_mlp.any():
            idx = np.asarray(results[c]["idx_out"]).reshape(2 * BPC)
            cnt_h = np.maximum(0.0, idx[BPC:] - idx[:BPC] + 1.0)
            gi = np.asarray(answer_idx)[:, sl].astype(np.float64)
            cnt_g = np.maximum(0.0, gi[1] - gi[0] + 1.0)
            ans[sl] += cnt_h[:, None].astype(np.float32) * b_mlp[None, :]
            goldo[sl] += cnt_g[:, None].astype(np.float32) * b_mlp[None, :]
    return ans, goldo


def kernel(doc_encoding, score_soft, answer_idx, W_mlp, b_mlp):
    nc = get_nc()
    in_maps = make_in_maps(doc_encoding, score_soft, answer_idx, W_mlp)
    res = run_bass_kernel_spmd(nc, in_maps, list(range(N_CORES))).results
    return assemble(res, answer_idx, b_mlp)
